# revision 26
# baseline (speedup 1.0000x reference)
"""Trainium2 Bass kernel for NLinear (per-feature batched Linear).

  out[b, n, o] = sum_i x[b, n, i] * W[n, i, o] + bias[n, o]
  x: [2048, 256, 256] f32, W: [256, 256, 256] f32, b: [256, 256] f32

Strategy: shard the feature dim n across 8 cores (32 features each, no
cross-device communication). Per core, for each feature n we compute
out_n.T = W_n.T @ x_n.T on the PE array with W stationary:

  psum[o_chunk(128), b_chunk(512)] += W[n, i_chunk, o_chunk] (as lhsT)
                                      @ xT[n, i_chunk, b_chunk]

Layout/precision choices:
- Host pre-casts x/W to bf16 and pre-transposes x to xT[n, i, b]
  (b contiguous) so every x DMA is one contiguous 512 KB block.
- The whole per-core W shard (4 MB bf16 = 32 KB/partition) is
  host-packed into SBUF lhsT layout [128, n*ic*o] and loaded once.
- PSUM accumulates fp32; bias is fused into the PSUM->SBUF evacuation
  (per-partition bias on the o axis) split across Scalar/Vector engines.
- Device writes outT[n, o, b] (bf16); host transposes back to [b, n, o].
"""

import numpy as np
import ml_dtypes

B = 2048
N = 256
D = 256  # d_in
O = 256  # d_out
NCORES = 8
NS = N // NCORES  # features per core

KC = D // 128   # contraction chunks (2)
OC = O // 128   # output-partition chunks (2)
BC = B // 512   # moving free-dim chunks (4)

OUT_BF16 = True  # device writes bf16 outputs (halves HBM write traffic)
REPEAT = 1  # benchmarking: run the whole kernel body this many times

_nc_cache = {}


def _build_nc():
    key = ("nc", OUT_BF16, REPEAT)
    if key in _nc_cache:
        return _nc_cache[key]

    import concourse.mybir as mybir
    from concourse import bacc
    from concourse.tile import TileContext

    bf16 = mybir.dt.bfloat16
    f32 = mybir.dt.float32
    out_dt = bf16 if OUT_BF16 else f32

    nc = bacc.Bacc(name="nlinear_spmd")
    # x pre-packed on host: xTh[n, p, ic*B + b] = x[b, n, ic*128 + p]
    # -> one fully-contiguous 1 MB DMA per feature n
    xT = nc.dram_tensor("xT", [NS, 128, KC * B], bf16, kind="ExternalInput")
    # W pre-packed on host to SBUF lhsT layout: Wh[p, ((n*KC+ic)*O + o)]
    Wh = nc.dram_tensor("Wh", [128, NS * KC * O], bf16, kind="ExternalInput")
    # bias pre-packed on host to SBUF layout: biash[p, (n*OC+oc)]
    biash = nc.dram_tensor("biash", [128, NS * OC], f32, kind="ExternalInput")
    outT = nc.dram_tensor("outT", [NS, O, B], out_dt, kind="ExternalOutput")

    PREFETCH = 6  # x-tiles loaded ahead of compute (per ic tag)

    def load_x(nc, xp, xtiles, n):
        xt = xp.tile([128, KC * B], bf16, name="x", tag="x")
        nc.sync.dma_start(xt, xT[n, :, :])
        xtiles[n] = xt

    def body(nc, xp, op, pp, wt_all, bias_t, xtiles, preload=False):
        if preload:
            for n in range(PREFETCH):
                load_x(nc, xp, xtiles, n)
        for n in range(NS):
            if n + PREFETCH < NS:
                load_x(nc, xp, xtiles, n + PREFETCH)
            xt = xtiles.pop(n)

            for oc in range(OC):
                ot = op.tile([128, B], out_dt, name="ot", tag="ot")
                pss = [
                    pp.tile([128, 512], f32, name="ps", tag="ps") for _ in range(BC)
                ]
                bias_ap = bias_t[:, n * OC + oc: n * OC + oc + 1]
                for bc in range(BC):
                    for ic in range(KC):
                        w_off = (n * KC + ic) * O + oc * 128
                        nc.tensor.matmul(
                            pss[bc],
                            wt_all[:, w_off:w_off + 128],
                            xt[:, ic * B + bc * 512:ic * B + (bc + 1) * 512],
                            start=(ic == 0),
                            stop=(ic == KC - 1),
                        )
                    dst = ot[:, bc * 512:(bc + 1) * 512]
                    if bc % 2 == 0:
                        nc.scalar.add(dst, pss[bc], bias_ap)
                    else:
                        nc.vector.tensor_scalar_add(dst, pss[bc], bias_ap)
                nc.scalar.dma_start(outT[n, oc * 128:(oc + 1) * 128, :], ot)

    with TileContext(nc) as tc:
        with (
            tc.tile_pool(name="xp", bufs=8) as xp,
            tc.tile_pool(name="op", bufs=6) as op,
            tc.tile_pool(name="cp", bufs=1) as cp,
            tc.tile_pool(name="pp", bufs=8, space="PSUM") as pp,
        ):
            bias_t = cp.tile([128, NS * OC], f32)
            nc.sync.dma_start(bias_t, biash[:, :])
            wt_all = cp.tile([128, NS * KC * O], bf16)
            # interleave W chunk loads with the first x prefetches so the
            # PE can start n=0 as soon as chunk 0 + x(0) land
            wcols = NS * KC * O // 4
            xtiles = {}
            if REPEAT > 1:
                for wc in range(4):
                    nc.sync.dma_start(
                        wt_all[:, wc * wcols:(wc + 1) * wcols],
                        Wh[:, wc * wcols:(wc + 1) * wcols],
                    )
                hint = tuple(
                    getattr(mybir.EngineType, e)
                    for e in ("PE", "Activation", "DVE", "SP", "Pool")
                )
                with tc.For_i(0, REPEAT, 1, hint_engines=hint):
                    body(nc, xp, op, pp, wt_all, bias_t, xtiles, preload=True)
            else:
                for wc in range(4):
                    nc.sync.dma_start(
                        wt_all[:, wc * wcols:(wc + 1) * wcols],
                        Wh[:, wc * wcols:(wc + 1) * wcols],
                    )
                    load_x(nc, xp, xtiles, wc)
                for n in range(4, PREFETCH):
                    load_x(nc, xp, xtiles, n)
                body(nc, xp, op, pp, wt_all, bias_t, xtiles)

    nc.finalize()
    _nc_cache[key] = nc
    return nc


def _run_spmd(nc, in_maps, **kwargs):
    from concourse import bass_utils

    return bass_utils.run_bass_kernel_spmd(
        nc, in_maps, core_ids=list(range(NCORES)), **kwargs
    )


def _pack_W(Wc):
    """[NS, D, O] (one core's W shard, bf16) -> [128, NS*KC*O] lhsT pack."""
    return np.ascontiguousarray(
        Wc.reshape(NS, KC, 128, O).transpose(2, 0, 1, 3).reshape(128, NS * KC * O)
    )


def kernel(x, W, b, **run_kwargs):
    nc = _build_nc()

    bf16 = ml_dtypes.bfloat16
    # [B, N, D] -> [N, 128(p), KC*B] with xT[n, p, ic*B+b] = x[b, n, ic*128+p]
    xT = np.ascontiguousarray(
        x.astype(bf16)
        .reshape(B, N, KC, 128)
        .transpose(1, 3, 2, 0)
        .reshape(N, 128, KC * B)
    )
    Wb = W.astype(bf16)
    bb = np.ascontiguousarray(b.astype(np.float32))

    in_maps = [
        {
            "xT": xT[c * NS:(c + 1) * NS],
            "Wh": _pack_W(Wb[c * NS:(c + 1) * NS]),
            # biash[p, n*OC+oc] = b[n, oc*128+p]
            "biash": np.ascontiguousarray(
                bb[c * NS:(c + 1) * NS]
                .reshape(NS, OC, 128)
                .transpose(2, 0, 1)
                .reshape(128, NS * OC)
            ),
        }
        for c in range(NCORES)
    ]
    res = _run_spmd(nc, in_maps, **run_kwargs)
    outT = np.concatenate([r["outT"] for r in res.results], axis=0)  # [N, O, B]
    out = np.ascontiguousarray(outT.astype(np.float32).transpose(2, 0, 1))
    if run_kwargs:
        kernel.last_result = res
    return out


# revision 27
# speedup vs baseline: 1.0272x; 1.0272x over previous
"""Trainium2 Bass kernel for NLinear (per-feature batched Linear).

  out[b, n, o] = sum_i x[b, n, i] * W[n, i, o] + bias[n, o]
  x: [2048, 256, 256] f32, W: [256, 256, 256] f32, b: [256, 256] f32

Strategy: shard the feature dim n across 8 cores (32 features each, no
cross-device communication). Per core, for each feature n we compute
out_n.T = W_n.T @ x_n.T on the PE array with W stationary:

  psum[o_chunk(128), b_chunk(512)] += W[n, i_chunk, o_chunk] (as lhsT)
                                      @ xT[n, i_chunk, b_chunk]

Layout/precision choices:
- Host pre-casts x/W to bf16 and pre-transposes x to xT[n, i, b]
  (b contiguous) so every x DMA is one contiguous 512 KB block.
- The whole per-core W shard (4 MB bf16 = 32 KB/partition) is
  host-packed into SBUF lhsT layout [128, n*ic*o] and loaded once.
- PSUM accumulates fp32; bias is fused into the PSUM->SBUF evacuation
  (per-partition bias on the o axis) split across Scalar/Vector engines.
- Device writes outT[n, o, b] (bf16); host transposes back to [b, n, o].
"""

import numpy as np
import ml_dtypes

B = 2048
N = 256
D = 256  # d_in
O = 256  # d_out
NCORES = 8
NS = N // NCORES  # features per core

KC = D // 128   # contraction chunks (2)
OC = O // 128   # output-partition chunks (2)
BC = B // 512   # moving free-dim chunks (4)

OUT_BF16 = True  # device writes bf16 outputs (halves HBM write traffic)
REPEAT = 1  # benchmarking: run the whole kernel body this many times

_nc_cache = {}


def _build_nc():
    key = ("nc", OUT_BF16, REPEAT)
    if key in _nc_cache:
        return _nc_cache[key]

    import concourse.mybir as mybir
    from concourse import bacc
    from concourse.tile import TileContext

    bf16 = mybir.dt.bfloat16
    f32 = mybir.dt.float32
    out_dt = bf16 if OUT_BF16 else f32

    nc = bacc.Bacc(name="nlinear_spmd")
    # x pre-packed on host: xTh[n, p, ic*B + b] = x[b, n, ic*128 + p]
    # -> one fully-contiguous 1 MB DMA per feature n
    xT = nc.dram_tensor("xT", [NS, 128, KC * B], bf16, kind="ExternalInput")
    # W pre-packed on host to SBUF lhsT layout: Wh[p, ((n*KC+ic)*O + o)]
    Wh = nc.dram_tensor("Wh", [128, NS * KC * O], bf16, kind="ExternalInput")
    # bias pre-packed on host to SBUF layout: biash[p, (n*OC+oc)]
    biash = nc.dram_tensor("biash", [128, NS * OC], f32, kind="ExternalInput")
    outT = nc.dram_tensor("outT", [NS, O, B], out_dt, kind="ExternalOutput")

    PREFETCH = 10  # x-tiles loaded ahead of compute (per ic tag)

    def load_x(nc, xp, xtiles, n):
        xt = xp.tile([128, KC * B], bf16, name="x", tag="x")
        nc.sync.dma_start(xt, xT[n, :, :])
        xtiles[n] = xt

    def body(nc, xp, op, pp, wt_all, bias_t, xtiles, preload=False):
        if preload:
            for n in range(PREFETCH):
                load_x(nc, xp, xtiles, n)
        for n in range(NS):
            if n + PREFETCH < NS:
                load_x(nc, xp, xtiles, n + PREFETCH)
            xt = xtiles.pop(n)

            for oc in range(OC):
                ot = op.tile([128, B], out_dt, name="ot", tag="ot")
                pss = [
                    pp.tile([128, 512], f32, name="ps", tag="ps") for _ in range(BC)
                ]
                bias_ap = bias_t[:, n * OC + oc: n * OC + oc + 1]
                for bc in range(BC):
                    for ic in range(KC):
                        w_off = (n * KC + ic) * O + oc * 128
                        nc.tensor.matmul(
                            pss[bc],
                            wt_all[:, w_off:w_off + 128],
                            xt[:, ic * B + bc * 512:ic * B + (bc + 1) * 512],
                            start=(ic == 0),
                            stop=(ic == KC - 1),
                        )
                    dst = ot[:, bc * 512:(bc + 1) * 512]
                    if bc % 2 == 0:
                        nc.scalar.add(dst, pss[bc], bias_ap)
                    else:
                        nc.vector.tensor_scalar_add(dst, pss[bc], bias_ap)
                nc.scalar.dma_start(outT[n, oc * 128:(oc + 1) * 128, :], ot)

    with TileContext(nc) as tc:
        with (
            tc.tile_pool(name="xp", bufs=11) as xp,
            tc.tile_pool(name="op", bufs=8) as op,
            tc.tile_pool(name="cp", bufs=1) as cp,
            tc.tile_pool(name="pp", bufs=8, space="PSUM") as pp,
        ):
            bias_t = cp.tile([128, NS * OC], f32)
            nc.sync.dma_start(bias_t, biash[:, :])
            wt_all = cp.tile([128, NS * KC * O], bf16)
            # interleave W chunk loads with the first x prefetches so the
            # PE can start n=0 as soon as chunk 0 + x(0) land
            wcols = NS * KC * O // 4
            xtiles = {}
            if REPEAT > 1:
                for wc in range(4):
                    nc.sync.dma_start(
                        wt_all[:, wc * wcols:(wc + 1) * wcols],
                        Wh[:, wc * wcols:(wc + 1) * wcols],
                    )
                hint = tuple(
                    getattr(mybir.EngineType, e)
                    for e in ("PE", "Activation", "DVE", "SP", "Pool")
                )
                with tc.For_i(0, REPEAT, 1, hint_engines=hint):
                    body(nc, xp, op, pp, wt_all, bias_t, xtiles, preload=True)
            else:
                for wc in range(4):
                    nc.sync.dma_start(
                        wt_all[:, wc * wcols:(wc + 1) * wcols],
                        Wh[:, wc * wcols:(wc + 1) * wcols],
                    )
                    load_x(nc, xp, xtiles, wc)
                for n in range(4, PREFETCH):
                    load_x(nc, xp, xtiles, n)
                body(nc, xp, op, pp, wt_all, bias_t, xtiles)

    nc.finalize()
    _nc_cache[key] = nc
    return nc


def _run_spmd(nc, in_maps, **kwargs):
    from concourse import bass_utils

    return bass_utils.run_bass_kernel_spmd(
        nc, in_maps, core_ids=list(range(NCORES)), **kwargs
    )


def _pack_W(Wc):
    """[NS, D, O] (one core's W shard, bf16) -> [128, NS*KC*O] lhsT pack."""
    return np.ascontiguousarray(
        Wc.reshape(NS, KC, 128, O).transpose(2, 0, 1, 3).reshape(128, NS * KC * O)
    )


def kernel(x, W, b, **run_kwargs):
    nc = _build_nc()

    bf16 = ml_dtypes.bfloat16
    # [B, N, D] -> [N, 128(p), KC*B] with xT[n, p, ic*B+b] = x[b, n, ic*128+p]
    xT = np.ascontiguousarray(
        x.astype(bf16)
        .reshape(B, N, KC, 128)
        .transpose(1, 3, 2, 0)
        .reshape(N, 128, KC * B)
    )
    Wb = W.astype(bf16)
    bb = np.ascontiguousarray(b.astype(np.float32))

    in_maps = [
        {
            "xT": xT[c * NS:(c + 1) * NS],
            "Wh": _pack_W(Wb[c * NS:(c + 1) * NS]),
            # biash[p, n*OC+oc] = b[n, oc*128+p]
            "biash": np.ascontiguousarray(
                bb[c * NS:(c + 1) * NS]
                .reshape(NS, OC, 128)
                .transpose(2, 0, 1)
                .reshape(128, NS * OC)
            ),
        }
        for c in range(NCORES)
    ]
    res = _run_spmd(nc, in_maps, **run_kwargs)
    outT = np.concatenate([r["outT"] for r in res.results], axis=0)  # [N, O, B]
    out = np.ascontiguousarray(outT.astype(np.float32).transpose(2, 0, 1))
    if run_kwargs:
        kernel.last_result = res
    return out


# revision 30
# speedup vs baseline: 1.0362x; 1.0087x over previous
"""Trainium2 Bass kernel for NLinear (per-feature batched Linear).

  out[b, n, o] = sum_i x[b, n, i] * W[n, i, o] + bias[n, o]
  x: [2048, 256, 256] f32, W: [256, 256, 256] f32, b: [256, 256] f32

Strategy: shard the feature dim n across 8 cores (32 features each, no
cross-device communication). Per core, for each feature n we compute
out_n.T = W_n.T @ x_n.T on the PE array with W stationary:

  psum[o_chunk(128), b_chunk(512)] += W[n, i_chunk, o_chunk] (as lhsT)
                                      @ xT[n, i_chunk, b_chunk]

Layout/precision choices:
- Host pre-casts x/W to bf16 and pre-transposes x to xT[n, i, b]
  (b contiguous) so every x DMA is one contiguous 512 KB block.
- The whole per-core W shard (4 MB bf16 = 32 KB/partition) is
  host-packed into SBUF lhsT layout [128, n*ic*o] and loaded once.
- PSUM accumulates fp32; bias is fused into the PSUM->SBUF evacuation
  (per-partition bias on the o axis) split across Scalar/Vector engines.
- Device writes outT[n, o, b] (bf16); host transposes back to [b, n, o].
"""

import numpy as np
import ml_dtypes

B = 2048
N = 256
D = 256  # d_in
O = 256  # d_out
NCORES = 8
NS = N // NCORES  # features per core

KC = D // 128   # contraction chunks (2)
OC = O // 128   # output-partition chunks (2)
BC = B // 512   # moving free-dim chunks (4)

OUT_BF16 = True  # device writes bf16 outputs (halves HBM write traffic)
REPEAT = 1  # benchmarking: run the whole kernel body this many times
STORE_RING = "scalar"  # which engine issues output-store DMAs


def STORE_ENGINE(nc):
    return getattr(nc, STORE_RING)

_nc_cache = {}


def _build_nc():
    key = ("nc", OUT_BF16, REPEAT, STORE_RING)
    if key in _nc_cache:
        return _nc_cache[key]

    import concourse.mybir as mybir
    from concourse import bacc
    from concourse.tile import TileContext

    bf16 = mybir.dt.bfloat16
    f32 = mybir.dt.float32
    out_dt = bf16 if OUT_BF16 else f32

    nc = bacc.Bacc(name="nlinear_spmd")
    # x pre-packed on host: xTh[n, p, ic*B + b] = x[b, n, ic*128 + p]
    # -> one fully-contiguous 1 MB DMA per feature n
    xT = nc.dram_tensor("xT", [NS, 128, KC * B], bf16, kind="ExternalInput")
    # W pre-packed on host to SBUF lhsT layout: Wh[p, ((n*KC+ic)*O + o)]
    Wh = nc.dram_tensor("Wh", [128, NS * KC * O], bf16, kind="ExternalInput")
    # bias pre-packed on host to SBUF layout: biash[p, (n*OC+oc)]
    biash = nc.dram_tensor("biash", [128, NS * OC], f32, kind="ExternalInput")
    outT = nc.dram_tensor("outT", [NS, O, B], out_dt, kind="ExternalOutput")

    PREFETCH = 10  # x-tiles loaded ahead of compute (per ic tag)

    def load_x(nc, xp, xtiles, n):
        xt = xp.tile([128, KC * B], bf16, name="x", tag="x")
        nc.sync.dma_start(xt, xT[n, :, :])
        xtiles[n] = xt

    def body(nc, xp, op, pp, wt_all, bias_t, xtiles, preload=False):
        if preload:
            for n in range(PREFETCH):
                load_x(nc, xp, xtiles, n)
        for n in range(NS):
            if n + PREFETCH < NS:
                load_x(nc, xp, xtiles, n + PREFETCH)
            xt = xtiles.pop(n)

            for oc in range(OC):
                ot = op.tile([128, B], out_dt, name="ot", tag="ot")
                pss = [
                    pp.tile([128, 512], f32, name="ps", tag="ps") for _ in range(BC)
                ]
                bias_ap = bias_t[:, n * OC + oc: n * OC + oc + 1]
                for bc in range(BC):
                    for ic in range(KC):
                        w_off = (n * KC + ic) * O + oc * 128
                        nc.tensor.matmul(
                            pss[bc],
                            wt_all[:, w_off:w_off + 128],
                            xt[:, ic * B + bc * 512:ic * B + (bc + 1) * 512],
                            start=(ic == 0),
                            stop=(ic == KC - 1),
                        )
                    dst = ot[:, bc * 512:(bc + 1) * 512]
                    if bc % 2 == 0:
                        nc.scalar.add(dst, pss[bc], bias_ap)
                    else:
                        nc.vector.tensor_scalar_add(dst, pss[bc], bias_ap)
                STORE_ENGINE(nc).dma_start(outT[n, oc * 128:(oc + 1) * 128, :], ot)

    with TileContext(nc) as tc:
        with (
            tc.tile_pool(name="xp", bufs=11) as xp,
            tc.tile_pool(name="op", bufs=8) as op,
            tc.tile_pool(name="cp", bufs=1) as cp,
            tc.tile_pool(name="pp", bufs=8, space="PSUM") as pp,
        ):
            bias_t = cp.tile([128, NS * OC], f32)
            nc.sync.dma_start(bias_t, biash[:, :])
            wt_all = cp.tile([128, NS * KC * O], bf16)
            # interleave W chunk loads with the first x prefetches so the
            # PE can start n=0 as soon as chunk 0 + x(0) land
            wcols = NS * KC * O // 4
            xtiles = {}
            if REPEAT > 1:
                for wc in range(4):
                    nc.sync.dma_start(
                        wt_all[:, wc * wcols:(wc + 1) * wcols],
                        Wh[:, wc * wcols:(wc + 1) * wcols],
                    )
                hint = tuple(
                    getattr(mybir.EngineType, e)
                    for e in ("PE", "Activation", "DVE", "SP", "Pool")
                )
                with tc.For_i(0, REPEAT, 1, hint_engines=hint):
                    body(nc, xp, op, pp, wt_all, bias_t, xtiles, preload=True)
            else:
                for wc in range(4):
                    nc.sync.dma_start(
                        wt_all[:, wc * wcols:(wc + 1) * wcols],
                        Wh[:, wc * wcols:(wc + 1) * wcols],
                    )
                    load_x(nc, xp, xtiles, wc)
                for n in range(4, PREFETCH):
                    load_x(nc, xp, xtiles, n)
                body(nc, xp, op, pp, wt_all, bias_t, xtiles)

    nc.finalize()
    _nc_cache[key] = nc
    return nc


def _run_spmd(nc, in_maps, **kwargs):
    from concourse import bass_utils

    return bass_utils.run_bass_kernel_spmd(
        nc, in_maps, core_ids=list(range(NCORES)), **kwargs
    )


def _pack_W(Wc):
    """[NS, D, O] (one core's W shard, bf16) -> [128, NS*KC*O] lhsT pack."""
    return np.ascontiguousarray(
        Wc.reshape(NS, KC, 128, O).transpose(2, 0, 1, 3).reshape(128, NS * KC * O)
    )


def kernel(x, W, b, **run_kwargs):
    nc = _build_nc()

    bf16 = ml_dtypes.bfloat16
    # [B, N, D] -> [N, 128(p), KC*B] with xT[n, p, ic*B+b] = x[b, n, ic*128+p]
    xT = np.ascontiguousarray(
        x.astype(bf16)
        .reshape(B, N, KC, 128)
        .transpose(1, 3, 2, 0)
        .reshape(N, 128, KC * B)
    )
    Wb = W.astype(bf16)
    bb = np.ascontiguousarray(b.astype(np.float32))

    in_maps = [
        {
            "xT": xT[c * NS:(c + 1) * NS],
            "Wh": _pack_W(Wb[c * NS:(c + 1) * NS]),
            # biash[p, n*OC+oc] = b[n, oc*128+p]
            "biash": np.ascontiguousarray(
                bb[c * NS:(c + 1) * NS]
                .reshape(NS, OC, 128)
                .transpose(2, 0, 1)
                .reshape(128, NS * OC)
            ),
        }
        for c in range(NCORES)
    ]
    res = _run_spmd(nc, in_maps, **run_kwargs)
    outT = np.concatenate([r["outT"] for r in res.results], axis=0)  # [N, O, B]
    out = np.ascontiguousarray(outT.astype(np.float32).transpose(2, 0, 1))
    if run_kwargs:
        kernel.last_result = res
    return out
